# revision 24
# baseline (speedup 1.0000x reference)
"""AutoQuantConv2d Trainium2 kernel.

Computes conv2d(fake_quant_nvfp4(x), fake_quant_nvfp4(w)) for
x [32,256,64,64] f32, w [256,256,3,3] f32, stride 1, pad 1, NCHW/OIHW.

Sharding: data-parallel over batch — each of the 8 NeuronCores gets 4
images and the full weight; outputs are concatenated on host.

On-core pipeline (v3 — PE-saturating schedule):
  1. NVFP4 fake-quant, exact fp32 bit arithmetic (no division):
       amax   = blocked absmax (16 contiguous elements)        [DVE reduce]
       scale  = 2*floor_pow2(max(amax/6, eps))                 [3 TS ops]
       q      = (v + t) - t,  t = max(v & 0x7f800000, scale) * 3*2^21
     The q line runs as ONE custom fused DVE op.  q is E2M1 x pow2 —
     exactly representable in fp8e5 (e5m2), so the matmul runs fp8
     DoubleRow with NO prescale/descale and no extra error.
  2. Quantized activations land in zero-padded [128,2,66,66] fp8e5
     image tiles (4 persistent tiles, one per image).
  3. Weights quantize to bf16 in [oc,(ic kh kw)] layout; all 36 lhsT
     [ic,2,oc] fp8e5 tiles come from PE-mode transposes (strided reads
     straight out of the quantized weights) + ScalarE cast evacs.
     oc0's run while the PE waits for the first activations; oc1's
     run in the n0 h1-quant bubble.
  4. conv2d as implicit GEMM, n-major: for image, oc-half, 8-row
     block quad: 9 taps x 4 PSUM banks of DoubleRow matmuls
     [K=256, M=128, N=512].  576 matmuls at the 219 ns issue floor.
  5. ScalarE evacuates PSUM -> SBUF and issues the output DMA on its
     own queue (no cross-queue waits); Sync queue carries only input
     DMAs so prefetch never blocks.
"""

import numpy as np

import concourse.bass as bass
import concourse.mybir as mybir
from concourse.tile import TileContext
from concourse.bass_utils import run_bass_kernel_spmd
from concourse import masks
from contextlib import ExitStack

AO = mybir.AluOpType
F32 = mybir.dt.float32
I32 = mybir.dt.int32
BF16 = mybir.dt.bfloat16
FP8 = mybir.dt.float8e5

N_CORES = 8
N_PER = 4          # images per core
C = 256            # input channels
O = 256            # output channels
H = W = 64
HP = WP = 66       # padded spatial
F = H * W          # 4096 pixels per channel
NB = F // 16       # 256 quant blocks per channel row
KF = C * 9         # 2304 flattened weight row per output channel
WNB = KF // 16     # 144 quant blocks per weight row
H0 = 34            # first-image row split (quant h0 rows)
FPLANE = 4368      # 66*66 padded to a multiple of 16

MASK_EXP = 0x7F800000
K_MAGIC = 6291456.0  # 3 * 2^21: t = floor_pow2(max(|v|,scale)) * K rounds to grid


# ---------------------------------------------------------------------------
# custom fused DVE op: q = (v + t) - t, t = max(v & expmask, scale) * K
# ---------------------------------------------------------------------------
def _get_fused_quant_op():
    from concourse.dve_ops import OPS, DveOp
    import concourse.dve_ops as dm
    from concourse.dve_spec import Spec, Src0, Src1, Bin, lower, maxx, _has_src1, C0, C1
    from concourse.dve_uop import DveOpSpec, AluOp

    name = "ANT_NVFP4_FUSED"
    for op in OPS:
        if op.name == name:
            return op
    t = Bin(AluOp.MULTIPLY, maxx(Bin(AluOp.BITWISE_AND, Src0, C0), Src1), C1)
    spec = Spec(
        body=Bin(AluOp.SUBTRACT, Bin(AluOp.ADD, Src0, t), t),
        reference=lambda in0, in1, s0, s1, imm2: in0,
    )
    shas = {}
    for ver in ("v3", "v4"):
        uops = lower(spec, ver=ver)
        shas[ver] = DveOpSpec(name=name, uops=uops, rd1_en=_has_src1(spec)).sha(ver)
    op = DveOp(name, spec, False, uops_sha=shas)
    OPS.append(op)
    dm._SUB_OPCODE_FOR_NAME[name] = dm._CUSTOM_DVE_ROW_BASE + len(OPS) - 1
    return op


def _split_waits(nc, maxw=1):
    """walrus here rejects >1 sync-wait per instruction; hoist extras onto
    preceding same-engine NOPs."""
    bbs = []
    for fn in nc.m.functions:
        for bb in fn.blocks:
            bbs.append((bb, list(bb.instructions)))
    new_lists = []
    for bb, insts in bbs:
        out = []
        for inst in insts:
            si = inst.sync_info
            waits = list(si.on_wait) if si and si.on_wait else []
            if len(waits) > maxw:
                chunks = [waits[i : i + maxw] for i in range(0, len(waits), maxw)]
                eng = nc.engines[inst.engine]
                for chunk in chunks[:-1]:
                    bi = eng.nop(nofuse=True)
                    ni = bi.ins if hasattr(bi, "ins") else bi
                    ni.sync_info = mybir.SyncInfo(on_wait=chunk, on_update=[])
                    out.append(ni)
                inst.sync_info = mybir.SyncInfo(
                    on_wait=chunks[-1], on_update=list(si.on_update or [])
                )
            out.append(inst)
        new_lists.append((bb, out))
    for bb, out in new_lists:
        bb.instructions = out


def _emit_quant(nc, qop, maskt, xd_ap, nblocks, amax, out_ap):
    """NVFP4 fake-quant of SBUF AP xd_ap [128, nblocks*16] f32 into out_ap."""
    nc.vector.tensor_reduce(
        amax[:, :],
        xd_ap.rearrange("p (b s) -> p b s", s=16),
        axis=mybir.AxisListType.X,
        op=AO.max,
        apply_absolute_value=True,
    )
    # scale bits = ((max(amax/6, eps)) & expmask) + 1<<23   (pow2, exact)
    nc.vector.tensor_scalar(amax[:, :], amax[:, :], 1.0 / 6.0, 6e-31, AO.mult, AO.max)
    am_i = amax[:, :].bitcast(I32)
    nc.vector.tensor_scalar(am_i, am_i, MASK_EXP, None, AO.bitwise_and)
    nc.vector.tensor_scalar(am_i, am_i, 0x00800000, None, AO.add)
    nc.vector._custom_dve(
        qop,
        out=out_ap,
        in0=xd_ap.rearrange("p (b s) -> p b s", s=16),
        in1=amax[:, :].broadcast_to([128, nblocks, 16]),
        s0=maskt[:, :],
        s1=K_MAGIC,
    )


def _build():
    qop = _get_fused_quant_op()
    nc = bass.Bass(trn_type="TRN2")
    x = nc.dram_tensor("x", [N_PER, C, H, W], F32, kind="ExternalInput")
    w = nc.dram_tensor("w", [O, C, 3, 3], F32, kind="ExternalInput")
    out = nc.dram_tensor("out", [N_PER, O, H, W], F32, kind="ExternalOutput")

    with TileContext(nc) as tc:
        with ExitStack() as ctx:
            wpool = ctx.enter_context(tc.tile_pool(name="wpool", bufs=1))
            lpool = ctx.enter_context(tc.tile_pool(name="lpool", bufs=1))
            xqpool = ctx.enter_context(tc.tile_pool(name="xqpool", bufs=1))
            xdpool = ctx.enter_context(tc.tile_pool(name="xdpool", bufs=4))
            smpool = ctx.enter_context(tc.tile_pool(name="smpool", bufs=3))
            obpool = ctx.enter_context(tc.tile_pool(name="obpool", bufs=8))
            pspool = ctx.enter_context(tc.tile_pool(name="ps", bufs=6, space="PSUM"))
            wtpool = ctx.enter_context(tc.tile_pool(name="wt", bufs=2, space="PSUM"))

            # ---- init constants (GpSimd) ----
            maskt = wpool.tile([128, 1], F32, name="maskt", tag="maskt")
            nc.gpsimd.memset(maskt[:, :].bitcast(I32), MASK_EXP)
            ident = wpool.tile([128, 128], BF16, name="ident", tag="ident")
            masks.make_identity(nc, ident[:, :])

            # ---- persistent per-image padded fp8 tiles; zero borders ----
            xq_tiles = []
            for n in range(N_PER):
                t = xqpool.tile([128, 2, FPLANE], FP8, name=f"xq{n}", tag=f"xq{n}")
                tv = t[:, :, 0 : HP * WP].rearrange("p c (h w) -> p c h w", h=HP)
                nc.gpsimd.memset(tv[:, :, 0, :], 0.0)
                nc.gpsimd.memset(tv[:, :, HP - 1, :], 0.0)
                nc.gpsimd.memset(tv[:, :, 1 : HP - 1, 0], 0.0)
                nc.gpsimd.memset(tv[:, :, 1 : HP - 1, WP - 1], 0.0)
                xq_tiles.append(t)

            # ---- input DMAs (Sync queue only), in priority order ----
            # w oc0 in ic-chunk halves so its quant can start earliest
            wf = {}
            wf[0] = wpool.tile([128, KF], F32, name="wf0", tag="wf0")
            for half in range(2):
                sl = slice(half * 1152, (half + 1) * 1152)
                nc.sync.dma_start(
                    out=wf[0][:, sl],
                    in_=w[0:128, :, :, :].rearrange("o i kh kw -> o (i kh kw)")[
                        :, sl
                    ],
                )
            # first image h0 row-halves (earliest first matmul)
            xd0 = {}
            for c in range(2):
                a = xdpool.tile([128, H0 * W], F32, name=f"xd0a_{c}", tag="xd0a", bufs=2)
                nc.sync.dma_start(
                    out=a[:, :],
                    in_=x[0, c * 128 : (c + 1) * 128, 0:H0, :].rearrange(
                        "c h w -> c (h w)"
                    ),
                )
                xd0[(c, 0)] = a
            # w oc1 next (needed for PE transposes in the n0 h1 bubble)
            wf[1] = wpool.tile([128, KF], F32, name="wf1", tag="wf1")
            nc.sync.dma_start(
                out=wf[1][:, :],
                in_=w[128:256, :, :, :].rearrange("o i kh kw -> o (i kh kw)"),
            )
            # first image h1
            for c in range(2):
                b = xdpool.tile(
                    [128, (H - H0) * W], F32, name=f"xd0b_{c}", tag="xd0b", bufs=2
                )
                nc.sync.dma_start(
                    out=b[:, :],
                    in_=x[0, c * 128 : (c + 1) * 128, H0:H, :].rearrange(
                        "c h w -> c (h w)"
                    ),
                )
                xd0[(c, 1)] = b
            # remaining images, full chunks
            xds = {}
            for n in range(1, N_PER):
                for c in range(2):
                    xd = xdpool.tile([128, F], F32, name=f"xd_{n}_{c}", tag="xd")
                    nc.sync.dma_start(
                        out=xd[:, :],
                        in_=x[n, c * 128 : (c + 1) * 128, :, :].rearrange(
                            "c h w -> c (h w)"
                        ),
                    )
                    xds[(n, c)] = xd

            # ---- weight quant (DVE) + PE transposes + ScalarE cast evacs ----
            lhsT = {}
            for oc in range(2):
                for kh in range(3):
                    for kw in range(3):
                        lhsT[(kh, kw, oc)] = lpool.tile(
                            [128, 2, 128], FP8,
                            name=f"l_{kh}{kw}{oc}", tag=f"l_{kh}{kw}{oc}",
                        )
            wqd = {}

            def emit_w_quant(oc, split):
                wqd[oc] = wpool.tile([128, KF], BF16, name=f"wqd{oc}", tag=f"wqd{oc}")
                if split:
                    for half in range(2):
                        sl = slice(half * 1152, (half + 1) * 1152)
                        wam = wpool.tile(
                            [128, 72], F32, name=f"wam{oc}_{half}",
                            tag=f"wam{oc}", bufs=2,
                        )
                        _emit_quant(nc, qop, maskt, wf[oc][:, sl], 72, wam, wqd[oc][:, sl])
                else:
                    wam = wpool.tile([128, WNB], F32, name=f"wam{oc}", tag=f"wam{oc}")
                    _emit_quant(nc, qop, maskt, wf[oc][:, :], WNB, wam, wqd[oc][:, :])

            def emit_w_transposes(oc):
                # ic-chunk major: chunk c's transposes unblock as soon as the
                # matching quant half lands
                wv = wqd[oc][:, :].rearrange("p (i k) -> p k i", k=9)
                for c in range(2):
                    for tap in range(9):
                        wtps = wtpool.tile(
                            [128, 128], BF16, name=f"wt{oc}_{c}_{tap}", tag="wt"
                        )
                        nc.tensor.transpose(
                            wtps[:, :], wv[:, tap, c * 128 : (c + 1) * 128], ident[:, :]
                        )
                        nc.scalar.activation(
                            lhsT[(tap // 3, tap % 3, oc)][:, c, :], wtps[:, :],
                            mybir.ActivationFunctionType.Copy,
                            scale=1.0,
                        )

            emit_w_quant(0, split=True)
            emit_w_transposes(0)
            # oc1's quant + transposes also run in the preamble: the PE is
            # idle until the first activations anyway, and this removes both
            # the mid-stream transpose slot and the h1-quant stall
            emit_w_quant(1, split=True)
            emit_w_transposes(1)

            # ---- first image h0 quant (both chunks) -> first matmuls ----
            xq0v = xq_tiles[0][:, :, 0 : HP * WP].rearrange("p c (h w) -> p c h w", h=HP)
            for c in range(2):
                am = smpool.tile(
                    [128, H0 * 4], F32, name=f"amax_0_{c}_h0", tag="amax0", bufs=2
                )
                _emit_quant(
                    nc, qop, maskt, xd0[(c, 0)][:, :], H0 * 4, am,
                    xq0v[:, c, 1 : H0 + 1, 1 : W + 1],
                )

            # ---- first image h1 quant ----
            for c in range(2):
                am = smpool.tile(
                    [128, (H - H0) * 4], F32, name=f"amax_0_{c}_h1", tag="amax1", bufs=2
                )
                _emit_quant(
                    nc, qop, maskt, xd0[(c, 1)][:, :], (H - H0) * 4, am,
                    xq0v[:, c, H0 + 1 : H + 1, 1 : W + 1],
                )

            def emit_n_quant(n):
                xqv = xq_tiles[n][:, :, 0 : HP * WP].rearrange(
                    "p c (h w) -> p c h w", h=HP
                )
                for c in range(2):
                    amax = smpool.tile([128, NB], F32, name=f"amax_{n}_{c}", tag="amax")
                    _emit_quant(
                        nc, qop, maskt, xds[(n, c)][:, :], NB, amax,
                        xqv[:, c, 1 : H + 1, 1 : W + 1],
                    )

            # ---- main matmul sweep: n-major, oc inner ----
            def emit_quad(n, oc, hq, xqv, evac_split=False):
                hbs = [hq * 4 + j for j in range(4)]
                pss = [
                    pspool.tile([128, 512], F32, name=f"ps_{n}_{oc}_{hb}", tag="ps")
                    for hb in hbs
                ]
                k = 0
                for kh in range(3):
                    for kw in range(3):
                        for j, hb in enumerate(hbs):
                            rhs = xqv[
                                :, :, hb * 8 + kh : hb * 8 + kh + 8, kw : kw + 64
                            ]
                            nc.tensor.matmul(
                                pss[j][:, :],
                                lhsT[(kh, kw, oc)][:, :, :],
                                rhs,
                                start=(k == 0),
                                stop=(k == 8),
                                perf_mode=mybir.MatmulPerfMode.DoubleRow,
                            )
                        k += 1
                for j, hb in enumerate(hbs):
                    ob = obpool.tile([128, 512], F32, name=f"ob_{n}_{oc}_{hb}", tag="ob")
                    dst = out[n, oc * 128 : (oc + 1) * 128, hb * 8 : hb * 8 + 8, :]
                    src = ob[:, :].rearrange("p (h w) -> p h w", h=8)
                    if evac_split and j % 2 == 1:
                        # DVE is idle by the last image; share evac + DMA
                        # issue so the tail drains ~2x faster
                        nc.vector.tensor_copy(ob[:, :], pss[j][:, :])
                        nc.gpsimd.dma_start(out=dst, in_=src)
                    else:
                        nc.scalar.activation(
                            ob[:, :], pss[j][:, :],
                            mybir.ActivationFunctionType.Copy,
                            scale=1.0,
                        )
                        nc.scalar.dma_start(out=dst, in_=src)

            for n in range(N_PER):
                xqv = xq_tiles[n][:, :, 0 : HP * WP].rearrange(
                    "p c (h w) -> p c h w", h=HP
                )
                for oc in range(2):
                    for hq in range(2):
                        emit_quad(n, oc, hq, xqv, evac_split=(n == 3))
                # n+1's quant is emitted after image n's quads so the Tile
                # scheduler can't hoist it over n0-critical DVE work
                if n + 1 < N_PER:
                    emit_n_quant(n + 1)

    mybir.codegen_inst_isa_subclasses(nc)
    _split_waits(nc, maxw=1)
    return nc


_NC_CACHE = None


def _get_nc():
    global _NC_CACHE
    if _NC_CACHE is None:
        _NC_CACHE = _build()
    return _NC_CACHE


def kernel(x: np.ndarray, w: np.ndarray) -> np.ndarray:
    x = np.ascontiguousarray(x, dtype=np.float32)
    w = np.ascontiguousarray(w, dtype=np.float32)
    nc = _get_nc()
    in_maps = [
        {"x": x[i * N_PER : (i + 1) * N_PER], "w": w} for i in range(N_CORES)
    ]
    res = run_bass_kernel_spmd(nc, in_maps, core_ids=list(range(N_CORES)))
    return np.concatenate([res.results[i]["out"] for i in range(N_CORES)], axis=0)


# revision 25
# speedup vs baseline: 1.1996x; 1.1996x over previous
"""AutoQuantConv2d Trainium2 kernel.

Computes conv2d(fake_quant_nvfp4(x), fake_quant_nvfp4(w)) for
x [32,256,64,64] f32, w [256,256,3,3] f32, stride 1, pad 1, NCHW/OIHW.

Sharding: data-parallel over batch — each of the 8 NeuronCores gets 4
images and the full weight; outputs are concatenated on host.

On-core pipeline (v3 — PE-saturating schedule):
  1. NVFP4 fake-quant, exact fp32 bit arithmetic (no division):
       amax   = blocked absmax (16 contiguous elements)        [DVE reduce]
       scale  = 2*floor_pow2(max(amax/6, eps))                 [3 TS ops]
       q      = (v + t) - t,  t = max(v & 0x7f800000, scale) * 3*2^21
     The q line runs as ONE custom fused DVE op.  q is E2M1 x pow2 —
     exactly representable in fp8e5 (e5m2), so the matmul runs fp8
     DoubleRow with NO prescale/descale and no extra error.
  2. Quantized activations land in zero-padded [128,2,66,66] fp8e5
     image tiles (4 persistent tiles, one per image).
  3. Weights quantize to bf16 in [oc,(ic kh kw)] layout; all 36 lhsT
     [ic,2,oc] fp8e5 tiles come from PE-mode transposes (strided reads
     straight out of the quantized weights) + ScalarE cast evacs.
     oc0's run while the PE waits for the first activations; oc1's
     run in the n0 h1-quant bubble.
  4. conv2d as implicit GEMM, n-major: for image, oc-half, 8-row
     block quad: 9 taps x 4 PSUM banks of DoubleRow matmuls
     [K=256, M=128, N=512].  576 matmuls at the 219 ns issue floor.
  5. ScalarE evacuates PSUM -> SBUF and issues the output DMA on its
     own queue (no cross-queue waits); Sync queue carries only input
     DMAs so prefetch never blocks.
"""

import numpy as np

import concourse.bass as bass
import concourse.mybir as mybir
from concourse.tile import TileContext
from concourse.bass_utils import run_bass_kernel_spmd
from concourse import masks
from contextlib import ExitStack

AO = mybir.AluOpType
F32 = mybir.dt.float32
I32 = mybir.dt.int32
BF16 = mybir.dt.bfloat16
FP8 = mybir.dt.float8e5

N_CORES = 8
N_PER = 4          # images per core
C = 256            # input channels
O = 256            # output channels
H = W = 64
HP = WP = 66       # padded spatial
F = H * W          # 4096 pixels per channel
NB = F // 16       # 256 quant blocks per channel row
KF = C * 9         # 2304 flattened weight row per output channel
WNB = KF // 16     # 144 quant blocks per weight row
H0 = 34            # first-image row split (quant h0 rows)
FPLANE = 4368      # 66*66 padded to a multiple of 16

MASK_EXP = 0x7F800000
K_MAGIC = 6291456.0  # 3 * 2^21: t = floor_pow2(max(|v|,scale)) * K rounds to grid


# ---------------------------------------------------------------------------
# custom fused DVE op: q = (v + t) - t, t = max(v & expmask, scale) * K
# ---------------------------------------------------------------------------
def _get_fused_quant_op():
    from concourse.dve_ops import OPS, DveOp
    import concourse.dve_ops as dm
    from concourse.dve_spec import Spec, Src0, Src1, Bin, lower, maxx, _has_src1, C0, C1
    from concourse.dve_uop import DveOpSpec, AluOp

    name = "ANT_NVFP4_FUSED"
    for op in OPS:
        if op.name == name:
            return op
    t = Bin(AluOp.MULTIPLY, maxx(Bin(AluOp.BITWISE_AND, Src0, C0), Src1), C1)
    spec = Spec(
        body=Bin(AluOp.SUBTRACT, Bin(AluOp.ADD, Src0, t), t),
        reference=lambda in0, in1, s0, s1, imm2: in0,
    )
    shas = {}
    for ver in ("v3", "v4"):
        uops = lower(spec, ver=ver)
        shas[ver] = DveOpSpec(name=name, uops=uops, rd1_en=_has_src1(spec)).sha(ver)
    op = DveOp(name, spec, False, uops_sha=shas)
    OPS.append(op)
    dm._SUB_OPCODE_FOR_NAME[name] = dm._CUSTOM_DVE_ROW_BASE + len(OPS) - 1
    return op


def _split_waits(nc, maxw=1):
    """walrus here rejects >1 sync-wait per instruction; hoist extras onto
    preceding same-engine NOPs."""
    bbs = []
    for fn in nc.m.functions:
        for bb in fn.blocks:
            bbs.append((bb, list(bb.instructions)))
    new_lists = []
    for bb, insts in bbs:
        out = []
        for inst in insts:
            si = inst.sync_info
            waits = list(si.on_wait) if si and si.on_wait else []
            if len(waits) > maxw:
                chunks = [waits[i : i + maxw] for i in range(0, len(waits), maxw)]
                eng = nc.engines[inst.engine]
                for chunk in chunks[:-1]:
                    bi = eng.nop(nofuse=True)
                    ni = bi.ins if hasattr(bi, "ins") else bi
                    ni.sync_info = mybir.SyncInfo(on_wait=chunk, on_update=[])
                    out.append(ni)
                inst.sync_info = mybir.SyncInfo(
                    on_wait=chunks[-1], on_update=list(si.on_update or [])
                )
            out.append(inst)
        new_lists.append((bb, out))
    for bb, out in new_lists:
        bb.instructions = out


def _emit_quant(nc, qop, maskt, xd_ap, nblocks, amax, out_ap):
    """NVFP4 fake-quant of SBUF AP xd_ap [128, nblocks*16] f32 into out_ap."""
    nc.vector.tensor_reduce(
        amax[:, :],
        xd_ap.rearrange("p (b s) -> p b s", s=16),
        axis=mybir.AxisListType.X,
        op=AO.max,
        apply_absolute_value=True,
    )
    # scale bits = ((max(amax/6, eps)) & expmask) + 1<<23   (pow2, exact)
    nc.vector.tensor_scalar(amax[:, :], amax[:, :], 1.0 / 6.0, 6e-31, AO.mult, AO.max)
    am_i = amax[:, :].bitcast(I32)
    nc.vector.tensor_scalar(am_i, am_i, MASK_EXP, None, AO.bitwise_and)
    nc.vector.tensor_scalar(am_i, am_i, 0x00800000, None, AO.add)
    nc.vector._custom_dve(
        qop,
        out=out_ap,
        in0=xd_ap.rearrange("p (b s) -> p b s", s=16),
        in1=amax[:, :].broadcast_to([128, nblocks, 16]),
        s0=maskt[:, :],
        s1=K_MAGIC,
    )


def _build():
    qop = _get_fused_quant_op()
    nc = bass.Bass(trn_type="TRN2")
    x = nc.dram_tensor("x", [N_PER, C, H, W], F32, kind="ExternalInput")
    w = nc.dram_tensor("w", [O, C, 3, 3], F32, kind="ExternalInput")
    out = nc.dram_tensor("out", [N_PER, O, H, W], F32, kind="ExternalOutput")

    with TileContext(nc) as tc:
        with ExitStack() as ctx:
            wpool = ctx.enter_context(tc.tile_pool(name="wpool", bufs=1))
            lpool = ctx.enter_context(tc.tile_pool(name="lpool", bufs=1))
            xqpool = ctx.enter_context(tc.tile_pool(name="xqpool", bufs=1))
            xdpool = ctx.enter_context(tc.tile_pool(name="xdpool", bufs=4))
            smpool = ctx.enter_context(tc.tile_pool(name="smpool", bufs=3))
            obpool = ctx.enter_context(tc.tile_pool(name="obpool", bufs=8))
            pspool = ctx.enter_context(tc.tile_pool(name="ps", bufs=6, space="PSUM"))
            wtpool = ctx.enter_context(tc.tile_pool(name="wt", bufs=2, space="PSUM"))

            # ---- init constants (GpSimd) ----
            maskt = wpool.tile([128, 1], F32, name="maskt", tag="maskt")
            nc.gpsimd.memset(maskt[:, :].bitcast(I32), MASK_EXP)
            ident = wpool.tile([128, 128], BF16, name="ident", tag="ident")
            masks.make_identity(nc, ident[:, :])

            # ---- persistent per-image padded fp8 tiles; zero borders ----
            xq_tiles = []
            for n in range(N_PER):
                t = xqpool.tile([128, 2, FPLANE], FP8, name=f"xq{n}", tag=f"xq{n}")
                tv = t[:, :, 0 : HP * WP].rearrange("p c (h w) -> p c h w", h=HP)
                nc.gpsimd.memset(tv[:, :, 0, :], 0.0)
                nc.gpsimd.memset(tv[:, :, HP - 1, :], 0.0)
                nc.gpsimd.memset(tv[:, :, 1 : HP - 1, 0], 0.0)
                nc.gpsimd.memset(tv[:, :, 1 : HP - 1, WP - 1], 0.0)
                xq_tiles.append(t)

            # ---- input DMAs (Sync queue only), in priority order ----
            # w oc0 in ic-chunk halves so its quant can start earliest
            wf = {}
            wf[0] = wpool.tile([128, KF], F32, name="wf0", tag="wf0")
            for half in range(2):
                sl = slice(half * 1152, (half + 1) * 1152)
                nc.sync.dma_start(
                    out=wf[0][:, sl],
                    in_=w[0:128, :, :, :].rearrange("o i kh kw -> o (i kh kw)")[
                        :, sl
                    ],
                )
            # first image h0 row-halves (earliest first matmul)
            xd0 = {}
            for c in range(2):
                a = xdpool.tile([128, H0 * W], F32, name=f"xd0a_{c}", tag="xd0a", bufs=2)
                nc.sync.dma_start(
                    out=a[:, :],
                    in_=x[0, c * 128 : (c + 1) * 128, 0:H0, :].rearrange(
                        "c h w -> c (h w)"
                    ),
                )
                xd0[(c, 0)] = a
            # w oc1 next (needed for PE transposes in the n0 h1 bubble)
            wf[1] = wpool.tile([128, KF], F32, name="wf1", tag="wf1")
            nc.sync.dma_start(
                out=wf[1][:, :],
                in_=w[128:256, :, :, :].rearrange("o i kh kw -> o (i kh kw)"),
            )
            # first image h1
            for c in range(2):
                b = xdpool.tile(
                    [128, (H - H0) * W], F32, name=f"xd0b_{c}", tag="xd0b", bufs=2
                )
                nc.sync.dma_start(
                    out=b[:, :],
                    in_=x[0, c * 128 : (c + 1) * 128, H0:H, :].rearrange(
                        "c h w -> c (h w)"
                    ),
                )
                xd0[(c, 1)] = b
            # remaining images, full chunks
            xds = {}
            for n in range(1, N_PER):
                for c in range(2):
                    xd = xdpool.tile([128, F], F32, name=f"xd_{n}_{c}", tag="xd")
                    nc.sync.dma_start(
                        out=xd[:, :],
                        in_=x[n, c * 128 : (c + 1) * 128, :, :].rearrange(
                            "c h w -> c (h w)"
                        ),
                    )
                    xds[(n, c)] = xd

            # ---- weight quant (DVE) + PE transposes + ScalarE cast evacs ----
            lhsT = {}
            for oc in range(2):
                for kh in range(3):
                    for kw in range(3):
                        lhsT[(kh, kw, oc)] = lpool.tile(
                            [128, 2, 128], FP8,
                            name=f"l_{kh}{kw}{oc}", tag=f"l_{kh}{kw}{oc}",
                        )
            wqd = {}

            def emit_w_quant(oc, split):
                wqd[oc] = wpool.tile([128, KF], BF16, name=f"wqd{oc}", tag=f"wqd{oc}")
                if split:
                    for half in range(2):
                        sl = slice(half * 1152, (half + 1) * 1152)
                        wam = wpool.tile(
                            [128, 72], F32, name=f"wam{oc}_{half}",
                            tag=f"wam{oc}", bufs=2,
                        )
                        _emit_quant(nc, qop, maskt, wf[oc][:, sl], 72, wam, wqd[oc][:, sl])
                else:
                    wam = wpool.tile([128, WNB], F32, name=f"wam{oc}", tag=f"wam{oc}")
                    _emit_quant(nc, qop, maskt, wf[oc][:, :], WNB, wam, wqd[oc][:, :])

            def emit_w_transposes(oc):
                # ic-chunk major: chunk c's transposes unblock as soon as the
                # matching quant half lands
                wv = wqd[oc][:, :].rearrange("p (i k) -> p k i", k=9)
                for c in range(2):
                    for tap in range(9):
                        wtps = wtpool.tile(
                            [128, 128], BF16, name=f"wt{oc}_{c}_{tap}", tag="wt"
                        )
                        nc.tensor.transpose(
                            wtps[:, :], wv[:, tap, c * 128 : (c + 1) * 128], ident[:, :]
                        )
                        nc.scalar.activation(
                            lhsT[(tap // 3, tap % 3, oc)][:, c, :], wtps[:, :],
                            mybir.ActivationFunctionType.Copy,
                            scale=1.0,
                        )

            emit_w_quant(0, split=True)
            emit_w_transposes(0)
            # oc1's quant + transposes also run in the preamble: the PE is
            # idle until the first activations anyway, and this removes both
            # the mid-stream transpose slot and the h1-quant stall
            emit_w_quant(1, split=True)
            emit_w_transposes(1)

            # ---- first image h0 quant (both chunks) -> first matmuls ----
            xq0v = xq_tiles[0][:, :, 0 : HP * WP].rearrange("p c (h w) -> p c h w", h=HP)
            for c in range(2):
                am = smpool.tile(
                    [128, H0 * 4], F32, name=f"amax_0_{c}_h0", tag="amax0", bufs=2
                )
                _emit_quant(
                    nc, qop, maskt, xd0[(c, 0)][:, :], H0 * 4, am,
                    xq0v[:, c, 1 : H0 + 1, 1 : W + 1],
                )

            # ---- first image h1 quant ----
            for c in range(2):
                am = smpool.tile(
                    [128, (H - H0) * 4], F32, name=f"amax_0_{c}_h1", tag="amax1", bufs=2
                )
                _emit_quant(
                    nc, qop, maskt, xd0[(c, 1)][:, :], (H - H0) * 4, am,
                    xq0v[:, c, H0 + 1 : H + 1, 1 : W + 1],
                )

            # ---- remaining images quant (n-order) ----
            for n in range(1, N_PER):
                xqv = xq_tiles[n][:, :, 0 : HP * WP].rearrange(
                    "p c (h w) -> p c h w", h=HP
                )
                for c in range(2):
                    amax = smpool.tile([128, NB], F32, name=f"amax_{n}_{c}", tag="amax")
                    _emit_quant(
                        nc, qop, maskt, xds[(n, c)][:, :], NB, amax,
                        xqv[:, c, 1 : H + 1, 1 : W + 1],
                    )

            # ---- main matmul sweep: n-major, oc inner ----
            def emit_quad(n, oc, hq, xqv):
                hbs = [hq * 4 + j for j in range(4)]
                pss = [
                    pspool.tile([128, 512], F32, name=f"ps_{n}_{oc}_{hb}", tag="ps")
                    for hb in hbs
                ]
                k = 0
                for kh in range(3):
                    for kw in range(3):
                        for j, hb in enumerate(hbs):
                            rhs = xqv[
                                :, :, hb * 8 + kh : hb * 8 + kh + 8, kw : kw + 64
                            ]
                            nc.tensor.matmul(
                                pss[j][:, :],
                                lhsT[(kh, kw, oc)][:, :, :],
                                rhs,
                                start=(k == 0),
                                stop=(k == 8),
                                perf_mode=mybir.MatmulPerfMode.DoubleRow,
                            )
                        k += 1
                for j, hb in enumerate(hbs):
                    ob = obpool.tile([128, 512], F32, name=f"ob_{n}_{oc}_{hb}", tag="ob")
                    nc.scalar.activation(
                        ob[:, :], pss[j][:, :],
                        mybir.ActivationFunctionType.Copy,
                        scale=1.0,
                    )
                    nc.scalar.dma_start(
                        out=out[n, oc * 128 : (oc + 1) * 128, hb * 8 : hb * 8 + 8, :],
                        in_=ob[:, :].rearrange("p (h w) -> p h w", h=8),
                    )

            for n in range(N_PER):
                xqv = xq_tiles[n][:, :, 0 : HP * WP].rearrange(
                    "p c (h w) -> p c h w", h=HP
                )
                for oc in range(2):
                    for hq in range(2):
                        emit_quad(n, oc, hq, xqv)

    mybir.codegen_inst_isa_subclasses(nc)
    _split_waits(nc, maxw=1)
    return nc


_NC_CACHE = None


def _get_nc():
    global _NC_CACHE
    if _NC_CACHE is None:
        _NC_CACHE = _build()
    return _NC_CACHE


def kernel(x: np.ndarray, w: np.ndarray) -> np.ndarray:
    x = np.ascontiguousarray(x, dtype=np.float32)
    w = np.ascontiguousarray(w, dtype=np.float32)
    nc = _get_nc()
    in_maps = [
        {"x": x[i * N_PER : (i + 1) * N_PER], "w": w} for i in range(N_CORES)
    ]
    res = run_bass_kernel_spmd(nc, in_maps, core_ids=list(range(N_CORES)))
    return np.concatenate([res.results[i]["out"] for i in range(N_CORES)], axis=0)


# revision 30
# speedup vs baseline: 1.2158x; 1.0136x over previous
"""AutoQuantConv2d Trainium2 kernel.

Computes conv2d(fake_quant_nvfp4(x), fake_quant_nvfp4(w)) for
x [32,256,64,64] f32, w [256,256,3,3] f32, stride 1, pad 1, NCHW/OIHW.

Sharding: data-parallel over batch — each of the 8 NeuronCores gets 4
images and the full weight; outputs are concatenated on host.

On-core pipeline (v3 — PE-saturating schedule):
  1. NVFP4 fake-quant, exact fp32 bit arithmetic (no division):
       amax   = blocked absmax (16 contiguous elements)        [DVE reduce]
       scale  = 2*floor_pow2(max(amax/6, eps))                 [3 TS ops]
       q      = (v + t) - t,  t = max(v & 0x7f800000, scale) * 3*2^21
     The q line runs as ONE custom fused DVE op.  q is E2M1 x pow2 —
     exactly representable in fp8e5 (e5m2), so the matmul runs fp8
     DoubleRow with NO prescale/descale and no extra error.
  2. Quantized activations land in zero-padded [128,2,66,66] fp8e5
     image tiles (4 persistent tiles, one per image).
  3. Weights quantize to bf16 in [oc,(ic kh kw)] layout; all 36 lhsT
     [ic,2,oc] fp8e5 tiles come from PE-mode transposes (strided reads
     straight out of the quantized weights) + ScalarE cast evacs.
     oc0's run while the PE waits for the first activations; oc1's
     run in the n0 h1-quant bubble.
  4. conv2d as implicit GEMM, n-major: for image, oc-half, 8-row
     block quad: 9 taps x 4 PSUM banks of DoubleRow matmuls
     [K=256, M=128, N=512].  576 matmuls at the 219 ns issue floor.
  5. ScalarE evacuates PSUM -> SBUF and issues the output DMA on its
     own queue (no cross-queue waits); Sync queue carries only input
     DMAs so prefetch never blocks.
"""

import numpy as np

import concourse.bass as bass
import concourse.mybir as mybir
from concourse.tile import TileContext
from concourse.bass_utils import run_bass_kernel_spmd
from concourse import masks
from contextlib import ExitStack

AO = mybir.AluOpType
F32 = mybir.dt.float32
I32 = mybir.dt.int32
BF16 = mybir.dt.bfloat16
FP8 = mybir.dt.float8e5

N_CORES = 8
N_PER = 4          # images per core
C = 256            # input channels
O = 256            # output channels
H = W = 64
HP = WP = 66       # padded spatial
F = H * W          # 4096 pixels per channel
NB = F // 16       # 256 quant blocks per channel row
KF = C * 9         # 2304 flattened weight row per output channel
WNB = KF // 16     # 144 quant blocks per weight row
H0 = 34            # first-image row split (quant h0 rows)
FPLANE = 4368      # 66*66 padded to a multiple of 16

MASK_EXP = 0x7F800000
K_MAGIC = 6291456.0  # 3 * 2^21: t = floor_pow2(max(|v|,scale)) * K rounds to grid


# ---------------------------------------------------------------------------
# custom fused DVE op: q = (v + t) - t, t = max(v & expmask, scale) * K
# ---------------------------------------------------------------------------
def _get_fused_quant_op():
    from concourse.dve_ops import OPS, DveOp
    import concourse.dve_ops as dm
    from concourse.dve_spec import Spec, Src0, Src1, Bin, lower, maxx, _has_src1, C0, C1
    from concourse.dve_uop import DveOpSpec, AluOp

    name = "ANT_NVFP4_FUSED2"
    for op in OPS:
        if op.name == name:
            return op
    # t = max(floor_pow2(|v|), floor_pow2(amax2)) * K with amax2 = 2*amax/6:
    # floor_pow2(2x) == 2*floor_pow2(x), so the +1<<23 scale doubling folds
    # into the in1 prescale and the whole scale bit-math runs in-op
    t = Bin(
        AluOp.MULTIPLY,
        maxx(
            Bin(AluOp.BITWISE_AND, Src0, C0),
            Bin(AluOp.BITWISE_AND, Src1, C0),
        ),
        C1,
    )
    spec = Spec(
        body=Bin(AluOp.SUBTRACT, Bin(AluOp.ADD, Src0, t), t),
        reference=lambda in0, in1, s0, s1, imm2: in0,
    )
    shas = {}
    for ver in ("v3", "v4"):
        uops = lower(spec, ver=ver)
        shas[ver] = DveOpSpec(name=name, uops=uops, rd1_en=_has_src1(spec)).sha(ver)
    op = DveOp(name, spec, False, uops_sha=shas)
    OPS.append(op)
    dm._SUB_OPCODE_FOR_NAME[name] = dm._CUSTOM_DVE_ROW_BASE + len(OPS) - 1
    return op


def _split_waits(nc, maxw=1):
    """walrus here rejects >1 sync-wait per instruction; hoist extras onto
    preceding same-engine NOPs."""
    bbs = []
    for fn in nc.m.functions:
        for bb in fn.blocks:
            bbs.append((bb, list(bb.instructions)))
    new_lists = []
    for bb, insts in bbs:
        out = []
        for inst in insts:
            si = inst.sync_info
            waits = list(si.on_wait) if si and si.on_wait else []
            if len(waits) > maxw:
                chunks = [waits[i : i + maxw] for i in range(0, len(waits), maxw)]
                eng = nc.engines[inst.engine]
                for chunk in chunks[:-1]:
                    bi = eng.nop(nofuse=True)
                    ni = bi.ins if hasattr(bi, "ins") else bi
                    ni.sync_info = mybir.SyncInfo(on_wait=chunk, on_update=[])
                    out.append(ni)
                inst.sync_info = mybir.SyncInfo(
                    on_wait=chunks[-1], on_update=list(si.on_update or [])
                )
            out.append(inst)
        new_lists.append((bb, out))
    for bb, out in new_lists:
        bb.instructions = out


def _emit_quant(nc, qop, maskt, xd_ap, nblocks, amax, out_ap):
    """NVFP4 fake-quant of SBUF AP xd_ap [128, nblocks*16] f32 into out_ap."""
    nc.vector.tensor_reduce(
        amax[:, :],
        xd_ap.rearrange("p (b s) -> p b s", s=16),
        axis=mybir.AxisListType.X,
        op=AO.max,
        apply_absolute_value=True,
    )
    # amax2 = max(amax*2/6, 2*eps); the fused op turns it into the pow2
    # scale via its in-op floor_pow2 (& expmask) stage
    nc.vector.tensor_scalar(amax[:, :], amax[:, :], 2.0 / 6.0, 1.2e-30, AO.mult, AO.max)
    nc.vector._custom_dve(
        qop,
        out=out_ap,
        in0=xd_ap.rearrange("p (b s) -> p b s", s=16),
        in1=amax[:, :].broadcast_to([128, nblocks, 16]),
        s0=maskt[:, :],
        s1=K_MAGIC,
    )


def _build():
    qop = _get_fused_quant_op()
    nc = bass.Bass(trn_type="TRN2")
    x = nc.dram_tensor("x", [N_PER, C, H, W], F32, kind="ExternalInput")
    w = nc.dram_tensor("w", [O, C, 3, 3], F32, kind="ExternalInput")
    out = nc.dram_tensor("out", [N_PER, O, H, W], F32, kind="ExternalOutput")

    with TileContext(nc) as tc:
        with ExitStack() as ctx:
            wpool = ctx.enter_context(tc.tile_pool(name="wpool", bufs=1))
            lpool = ctx.enter_context(tc.tile_pool(name="lpool", bufs=1))
            xqpool = ctx.enter_context(tc.tile_pool(name="xqpool", bufs=1))
            xdpool = ctx.enter_context(tc.tile_pool(name="xdpool", bufs=4))
            smpool = ctx.enter_context(tc.tile_pool(name="smpool", bufs=3))
            obpool = ctx.enter_context(tc.tile_pool(name="obpool", bufs=8))
            pspool = ctx.enter_context(tc.tile_pool(name="ps", bufs=6, space="PSUM"))
            wtpool = ctx.enter_context(tc.tile_pool(name="wt", bufs=2, space="PSUM"))

            # ---- init constants (GpSimd) ----
            maskt = wpool.tile([128, 1], F32, name="maskt", tag="maskt")
            nc.gpsimd.memset(maskt[:, :].bitcast(I32), MASK_EXP)
            ident = wpool.tile([128, 128], BF16, name="ident", tag="ident")
            masks.make_identity(nc, ident[:, :])

            # ---- persistent per-image padded fp8 tiles; zero borders ----
            xq_tiles = []
            for n in range(N_PER):
                t = xqpool.tile([128, 2, FPLANE], FP8, name=f"xq{n}", tag=f"xq{n}")
                tv = t[:, :, 0 : HP * WP].rearrange("p c (h w) -> p c h w", h=HP)
                nc.gpsimd.memset(tv[:, :, 0, :], 0.0)
                nc.gpsimd.memset(tv[:, :, HP - 1, :], 0.0)
                nc.gpsimd.memset(tv[:, :, 1 : HP - 1, 0], 0.0)
                nc.gpsimd.memset(tv[:, :, 1 : HP - 1, WP - 1], 0.0)
                xq_tiles.append(t)

            # ---- input DMAs (Sync queue only), in priority order ----
            # w oc0 in ic-chunk halves so its quant can start earliest
            wf = {}
            wf[0] = wpool.tile([128, KF], F32, name="wf0", tag="wf0")
            for half in range(2):
                sl = slice(half * 1152, (half + 1) * 1152)
                nc.sync.dma_start(
                    out=wf[0][:, sl],
                    in_=w[0:128, :, :, :].rearrange("o i kh kw -> o (i kh kw)")[
                        :, sl
                    ],
                )
            # first image h0 row-halves (earliest first matmul)
            xd0 = {}
            for c in range(2):
                a = xdpool.tile([128, H0 * W], F32, name=f"xd0a_{c}", tag="xd0a", bufs=2)
                nc.sync.dma_start(
                    out=a[:, :],
                    in_=x[0, c * 128 : (c + 1) * 128, 0:H0, :].rearrange(
                        "c h w -> c (h w)"
                    ),
                )
                xd0[(c, 0)] = a
            # w oc1 next (needed for PE transposes in the n0 h1 bubble)
            wf[1] = wpool.tile([128, KF], F32, name="wf1", tag="wf1")
            nc.sync.dma_start(
                out=wf[1][:, :],
                in_=w[128:256, :, :, :].rearrange("o i kh kw -> o (i kh kw)"),
            )
            # first image h1
            for c in range(2):
                b = xdpool.tile(
                    [128, (H - H0) * W], F32, name=f"xd0b_{c}", tag="xd0b", bufs=2
                )
                nc.sync.dma_start(
                    out=b[:, :],
                    in_=x[0, c * 128 : (c + 1) * 128, H0:H, :].rearrange(
                        "c h w -> c (h w)"
                    ),
                )
                xd0[(c, 1)] = b
            # remaining images, full chunks
            xds = {}
            for n in range(1, N_PER):
                for c in range(2):
                    xd = xdpool.tile([128, F], F32, name=f"xd_{n}_{c}", tag="xd")
                    nc.sync.dma_start(
                        out=xd[:, :],
                        in_=x[n, c * 128 : (c + 1) * 128, :, :].rearrange(
                            "c h w -> c (h w)"
                        ),
                    )
                    xds[(n, c)] = xd

            # ---- weight quant (DVE) + PE transposes + ScalarE cast evacs ----
            lhsT = {}
            for oc in range(2):
                for kh in range(3):
                    for kw in range(3):
                        lhsT[(kh, kw, oc)] = lpool.tile(
                            [128, 2, 128], FP8,
                            name=f"l_{kh}{kw}{oc}", tag=f"l_{kh}{kw}{oc}",
                        )
            wqd = {}

            def emit_w_quant(oc, split):
                wqd[oc] = wpool.tile([128, KF], BF16, name=f"wqd{oc}", tag=f"wqd{oc}")
                if split:
                    for half in range(2):
                        sl = slice(half * 1152, (half + 1) * 1152)
                        wam = wpool.tile(
                            [128, 72], F32, name=f"wam{oc}_{half}",
                            tag=f"wam{oc}", bufs=2,
                        )
                        _emit_quant(nc, qop, maskt, wf[oc][:, sl], 72, wam, wqd[oc][:, sl])
                else:
                    wam = wpool.tile([128, WNB], F32, name=f"wam{oc}", tag=f"wam{oc}")
                    _emit_quant(nc, qop, maskt, wf[oc][:, :], WNB, wam, wqd[oc][:, :])

            def emit_w_transposes(oc):
                # ic-chunk major: chunk c's transposes unblock as soon as the
                # matching quant half lands
                wv = wqd[oc][:, :].rearrange("p (i k) -> p k i", k=9)
                for c in range(2):
                    for tap in range(9):
                        wtps = wtpool.tile(
                            [128, 128], BF16, name=f"wt{oc}_{c}_{tap}", tag="wt"
                        )
                        nc.tensor.transpose(
                            wtps[:, :], wv[:, tap, c * 128 : (c + 1) * 128], ident[:, :]
                        )
                        nc.scalar.activation(
                            lhsT[(tap // 3, tap % 3, oc)][:, c, :], wtps[:, :],
                            mybir.ActivationFunctionType.Copy,
                            scale=1.0,
                        )

            emit_w_quant(0, split=False)
            emit_w_transposes(0)
            # oc1's quant + transposes also run in the preamble: the PE is
            # idle until the first activations anyway, and this removes both
            # the mid-stream transpose slot and the h1-quant stall
            emit_w_quant(1, split=False)
            emit_w_transposes(1)

            # ---- first image h0 quant (both chunks) -> first matmuls ----
            xq0v = xq_tiles[0][:, :, 0 : HP * WP].rearrange("p c (h w) -> p c h w", h=HP)
            for c in range(2):
                am = smpool.tile(
                    [128, H0 * 4], F32, name=f"amax_0_{c}_h0", tag="amax0", bufs=2
                )
                _emit_quant(
                    nc, qop, maskt, xd0[(c, 0)][:, :], H0 * 4, am,
                    xq0v[:, c, 1 : H0 + 1, 1 : W + 1],
                )

            # ---- first image h1 quant ----
            for c in range(2):
                am = smpool.tile(
                    [128, (H - H0) * 4], F32, name=f"amax_0_{c}_h1", tag="amax1", bufs=2
                )
                _emit_quant(
                    nc, qop, maskt, xd0[(c, 1)][:, :], (H - H0) * 4, am,
                    xq0v[:, c, H0 + 1 : H + 1, 1 : W + 1],
                )

            # ---- remaining images quant (n-order) ----
            for n in range(1, N_PER):
                xqv = xq_tiles[n][:, :, 0 : HP * WP].rearrange(
                    "p c (h w) -> p c h w", h=HP
                )
                for c in range(2):
                    amax = smpool.tile([128, NB], F32, name=f"amax_{n}_{c}", tag="amax")
                    _emit_quant(
                        nc, qop, maskt, xds[(n, c)][:, :], NB, amax,
                        xqv[:, c, 1 : H + 1, 1 : W + 1],
                    )

            # ---- main matmul sweep: n-major, oc inner ----
            def emit_quad(n, oc, hq, xqv, evac_split=False):
                hbs = [hq * 4 + j for j in range(4)]
                pss = [
                    pspool.tile([128, 512], F32, name=f"ps_{n}_{oc}_{hb}", tag="ps")
                    for hb in hbs
                ]
                k = 0
                for kh in range(3):
                    for kw in range(3):
                        for j, hb in enumerate(hbs):
                            rhs = xqv[
                                :, :, hb * 8 + kh : hb * 8 + kh + 8, kw : kw + 64
                            ]
                            nc.tensor.matmul(
                                pss[j][:, :],
                                lhsT[(kh, kw, oc)][:, :, :],
                                rhs,
                                start=(k == 0),
                                stop=(k == 8),
                                perf_mode=mybir.MatmulPerfMode.DoubleRow,
                            )
                        k += 1
                for j, hb in enumerate(hbs):
                    ob = obpool.tile([128, 512], F32, name=f"ob_{n}_{oc}_{hb}", tag="ob")
                    dst = out[n, oc * 128 : (oc + 1) * 128, hb * 8 : hb * 8 + 8, :]
                    src = ob[:, :].rearrange("p (h w) -> p h w", h=8)
                    if evac_split and j % 2 == 1:
                        # DVE is idle by the last image; share evac + DMA
                        # issue so the closing quads drain ~2x faster
                        nc.vector.tensor_copy(ob[:, :], pss[j][:, :])
                        nc.gpsimd.dma_start(out=dst, in_=src)
                    else:
                        nc.scalar.activation(
                            ob[:, :], pss[j][:, :],
                            mybir.ActivationFunctionType.Copy,
                            scale=1.0,
                        )
                        nc.scalar.dma_start(out=dst, in_=src)

            for n in range(N_PER):
                xqv = xq_tiles[n][:, :, 0 : HP * WP].rearrange(
                    "p c (h w) -> p c h w", h=HP
                )
                for oc in range(2):
                    for hq in range(2):
                        emit_quad(n, oc, hq, xqv, evac_split=(n == 3))

    mybir.codegen_inst_isa_subclasses(nc)
    _split_waits(nc, maxw=1)
    return nc


_NC_CACHE = None


def _get_nc():
    global _NC_CACHE
    if _NC_CACHE is None:
        _NC_CACHE = _build()
    return _NC_CACHE


def kernel(x: np.ndarray, w: np.ndarray) -> np.ndarray:
    x = np.ascontiguousarray(x, dtype=np.float32)
    w = np.ascontiguousarray(w, dtype=np.float32)
    nc = _get_nc()
    in_maps = [
        {"x": x[i * N_PER : (i + 1) * N_PER], "w": w} for i in range(N_CORES)
    ]
    res = run_bass_kernel_spmd(nc, in_maps, core_ids=list(range(N_CORES)))
    return np.concatenate([res.results[i]["out"] for i in range(N_CORES)], axis=0)
